# revision 1
# baseline (speedup 1.0000x reference)
"""DeepFM Trainium2 kernel — 8-core SPMD, batch-sharded.

Strategy: shard the batch (16384 -> 8 x 2048); replicate the (host-packed,
bf16) embedding table and MLP weights on every core.  Per core:
  - indirect-DMA gather of 65-bf16-element rows ([64 emb | 1 lin]) from a
    combined [F*V, 96]-strided table, sample-major in SBUF
  - PE transposes to feature-major h^T [1728, batch]
  - bf16 MLP (1728->1024->512->256->1) feature-major, fp32 PSUM accumulate
  - FM second order via stacked-identity matmuls: A=sum_f e, B=sum_f e^2,
    second = 0.5*sum_d(A^2 - B) folded into the final 1-row matmul
  - first order: fp32 matmul of [Wd|Wld] against dense^T; lin-sum and
    bias/bld folded into the final matmul's contraction rows
"""

import numpy as np
import ml_dtypes

B, F, V, D, ND = 16384, 26, 100000, 64, 13
H1, H2, H3 = 1024, 512, 256
NCORES = 8
BS = B // NCORES            # 2048 samples per core
SUB = 128                   # gather sub-tile (samples)
NT = 512                    # outer batch tile (matmul N)
NOUT = BS // NT             # 4 outer tiles per core
NSUB = NT // SUB            # 4 sub-tiles per outer tile
ROWE = D + 1                # gathered elements per row (64 emb + 1 lin)
RSTRIDE = 96                # table row stride in elements (192B, 64B-aligned)
FV = F * V
HTOT = (F + 1) * D          # 1728
NCH = 14                    # ceil(1728/128); chunk 13 has 64 rows
KFIN = 97                   # final misc matmul contraction: 64 C + 26 lin + pad + bias@96

_cache = {}


def _build_nc(reps=1):
    import concourse.bass as bass
    import concourse.bacc as bacc
    import concourse.mybir as mybir
    import concourse.tile as tile

    dt = mybir.dt
    nc = bacc.Bacc()

    denseT = nc.declare_dram_parameter("denseT", [ND, BS], dt.float32, isOutput=False)
    idx = nc.declare_dram_parameter("idx", [BS, F], dt.int32, isOutput=False)
    table = nc.declare_dram_parameter("table", [FV, RSTRIDE], dt.bfloat16, isOutput=False)
    wdcat = nc.declare_dram_parameter("wdcat", [ND, D], dt.float32, isOutput=False)
    wld = nc.declare_dram_parameter("wld", [ND, 1], dt.float32, isOutput=False)
    w1 = nc.declare_dram_parameter("w1", [HTOT, H1], dt.bfloat16, isOutput=False)
    w2 = nc.declare_dram_parameter("w2", [H1, H2], dt.bfloat16, isOutput=False)
    w3 = nc.declare_dram_parameter("w3", [H2, H3], dt.bfloat16, isOutput=False)
    wout = nc.declare_dram_parameter("wout", [H3, 1], dt.bfloat16, isOutput=False)
    coeff = nc.declare_dram_parameter("coeff", [128, 1], dt.float32, isOutput=False)
    stacki = nc.declare_dram_parameter("stacki", [128, D], dt.bfloat16, isOutput=False)
    ident = nc.declare_dram_parameter("ident", [128, 128], dt.bfloat16, isOutput=False)
    out = nc.declare_dram_parameter("out", [NOUT, NT], dt.float32, isOutput=True)

    with tile.TileContext(nc) as tc:
        with (
            tc.tile_pool(name="const", bufs=1) as constp,
            tc.tile_pool(name="g", bufs=8) as gp,
            tc.tile_pool(name="ht", bufs=2) as htp,
            tc.tile_pool(name="act", bufs=2) as actp,
            tc.tile_pool(name="sq", bufs=3) as sqp,
            tc.tile_pool(name="misc", bufs=2) as miscp,
            tc.tile_pool(name="pdcat", bufs=1, space="PSUM") as pdcatp,
            tc.tile_pool(name="ptr", bufs=2, space="PSUM") as ptrp,
            tc.tile_pool(name="pab", bufs=1, space="PSUM") as pabp,
            tc.tile_pool(name="pfin", bufs=1, space="PSUM") as pfinp,
            tc.tile_pool(name="pl", bufs=2, space="PSUM") as plp,
        ):
            # ---- constants / weights, loaded once ----
            densesb = constp.tile([ND, BS], dt.float32)
            nc.sync.dma_start(out=densesb[:], in_=denseT[:])
            idxsb = constp.tile([SUB, (BS // SUB) * F], dt.int32)
            nc.sync.dma_start(
                out=idxsb[:].rearrange("p (st f) -> p st f", f=F),
                in_=idx[:].rearrange("(st p) f -> p st f", p=SUB),
            )
            wdcatsb = constp.tile([ND, D], dt.float32)
            nc.sync.dma_start(out=wdcatsb[:], in_=wdcat[:])
            wldsb = constp.tile([ND, 1], dt.float32)
            nc.sync.dma_start(out=wldsb[:], in_=wld[:])
            w1sb = constp.tile([128, NCH * H1], dt.bfloat16)
            for c in range(NCH):
                kc = min(128, HTOT - c * 128)
                nc.sync.dma_start(
                    out=w1sb[:kc, c * H1:(c + 1) * H1],
                    in_=w1[c * 128:c * 128 + kc, :],
                )
            w2sb = constp.tile([128, (H1 // 128) * H2], dt.bfloat16)
            for c in range(H1 // 128):
                nc.sync.dma_start(
                    out=w2sb[:, c * H2:(c + 1) * H2],
                    in_=w2[c * 128:(c + 1) * 128, :],
                )
            w3sb = constp.tile([128, (H2 // 128) * H3], dt.bfloat16)
            for c in range(H2 // 128):
                nc.sync.dma_start(
                    out=w3sb[:, c * H3:(c + 1) * H3],
                    in_=w3[c * 128:(c + 1) * 128, :],
                )
            woutsb = constp.tile([128, H3 // 128], dt.bfloat16)
            nc.sync.dma_start(
                out=woutsb[:], in_=wout[:].rearrange("(c p) one -> p (c one)", p=128)
            )
            coeffsb = constp.tile([128, 1], dt.float32)
            nc.sync.dma_start(out=coeffsb[:], in_=coeff[:])
            stackisb = constp.tile([128, D], dt.bfloat16)
            nc.sync.dma_start(out=stackisb[:], in_=stacki[:])
            identsb = constp.tile([128, 128], dt.bfloat16)
            nc.sync.dma_start(out=identsb[:], in_=ident[:])

            # ================= software-pipelined tile loop =================
            # Stage k runs the "front" of tile cur (gathers, dense matmul,
            # PE transposes to feature-major) interleaved with the "compute"
            # of tile prev (FM sums + MLP + final row), so the PE never has
            # long idle gaps (keeps HAM at 8/8) and DVE copies hide under
            # matmul phases.
            tiles = [t for _ in range(reps) for t in range(NOUT)]
            steps = [(tiles[k], tiles[k - 1] if k else None) for k in range(len(tiles))]
            steps.append((None, tiles[-1]))
            H = {}   # live handles for the in-flight tile

            def chunk_feats(c):
                if c == 0:
                    return D, D, [(D, 0)]
                if c == NCH - 1:
                    return 0, D, [(0, F - 1)]
                return 0, 128, [(0, 2 * c - 1), (D, 2 * c)]

            for cur, prev in steps:
                P = H.get(prev)
                if cur is not None:
                    # emb rows gathered feature-contiguous (64 elems) so a
                    # feature PAIR is one contiguous [128, 128] block; lin
                    # values gathered separately (element_offset=64)
                    gtiles = [gp.tile([SUB, F * D], dt.bfloat16, tag="g",
                                      name=f"g{s}")
                              for s in range(NSUB)]
                    gl = gp.tile([SUB, NSUB * F], dt.bfloat16, tag="gl")
                    for f in range(F):
                        for s in range(NSUB):
                            st = cur * NSUB + s
                            nc.gpsimd.indirect_dma_start(
                                out=gtiles[s][:, f * D:(f + 1) * D],
                                out_offset=None,
                                in_=table[:],
                                in_offset=bass.IndirectOffsetOnAxis(
                                    ap=idxsb[:, st * F + f:st * F + f + 1], axis=0
                                ),
                            )
                    for f in range(F):
                        for s in range(NSUB):
                            st = cur * NSUB + s
                            nc.gpsimd.indirect_dma_start(
                                out=gl[:, s * F + f:s * F + f + 1],
                                out_offset=None,
                                in_=table[:],
                                in_offset=bass.IndirectOffsetOnAxis(
                                    ap=idxsb[:, st * F + f:st * F + f + 1], axis=0
                                ),
                                element_offset=D,
                            )
                    pdcat = pdcatp.tile([D, NT], dt.float32)
                    nc.tensor.matmul(
                        out=pdcat[:],
                        lhsT=wdcatsb[:],
                        rhs=densesb[:, cur * NT:(cur + 1) * NT],
                        start=True,
                        stop=True,
                    )
                    ht = htp.tile([128, NCH * NT], dt.bfloat16, tag="ht")
                    nc.scalar.activation(
                        out=ht[0:D, 0:NT],
                        in_=pdcat[0:D, :],
                        func=mybir.ActivationFunctionType.Copy,
                    )
                    cext = miscp.tile([128, NT], dt.float32, tag="cext")
                    nc.vector.memset(cext[D:128, :], 0.0)
                    nc.vector.memset(cext[96:97, :], 1.0)
                    C = {"g": gtiles, "gl": gl, "ht": ht, "cext": cext}
                    H[cur] = C
                if P is not None:
                    pa = pabp.tile([D, NT], dt.float32, tag="pa")
                    pb = pabp.tile([D, NT], dt.float32, tag="pb")
                    h1t = actp.tile([128, (H1 // 128) * NT], dt.bfloat16, tag="h1t")

                # interleaved: per chunk c, prev's sq/A/B + cur's transposes,
                # plus one L1 m-tile for the first 8 chunks
                for c in range(NCH):
                    kc = min(128, HTOT - c * 128)
                    if P is not None:
                        htc = P["ht"][0:kc, c * NT:(c + 1) * NT]
                        sq = sqp.tile([128, NT], dt.bfloat16, tag="sq")
                        nc.vector.tensor_tensor(
                            out=sq[0:kc, :], in0=htc, in1=htc,
                            op=mybir.AluOpType.mult,
                        )
                        nc.tensor.matmul(
                            out=pa[:], lhsT=stackisb[0:kc, :], rhs=htc,
                            start=(c == 0), stop=(c == NCH - 1),
                        )
                        nc.tensor.matmul(
                            out=pb[:], lhsT=stackisb[0:kc, :], rhs=sq[0:kc, :],
                            start=(c == 0), stop=(c == NCH - 1),
                        )
                    if cur is not None:
                        # transpose via REGULAR matmul G.T @ I — unlike
                        # transpose-mode this counts as PE-busy for HAM
                        plo, kcc, feats = chunk_feats(c)
                        f0 = feats[0][1]
                        ptr = ptrp.tile([128, NT], dt.float32, tag="ptr")
                        for s in range(NSUB):
                            nc.tensor.matmul(
                                out=ptr[plo:plo + kcc, s * SUB:(s + 1) * SUB],
                                lhsT=C["g"][s][:, f0 * D:f0 * D + kcc],
                                rhs=identsb[:],
                                start=True,
                                stop=True,
                            )
                        nc.vector.tensor_copy(
                            out=C["ht"][plo:plo + kcc, c * NT:(c + 1) * NT],
                            in_=ptr[plo:plo + kcc, :],
                        )
                    if P is not None and c < H1 // 128:
                        m = c
                        pl = plp.tile([128, NT], dt.float32, tag="pl")
                        for cc in range(NCH):
                            kcc2 = min(128, HTOT - cc * 128)
                            nc.tensor.matmul(
                                out=pl[:],
                                lhsT=w1sb[0:kcc2, cc * H1 + m * 128:cc * H1 + (m + 1) * 128],
                                rhs=P["ht"][0:kcc2, cc * NT:(cc + 1) * NT],
                                start=(cc == 0),
                                stop=(cc == NCH - 1),
                            )
                        nc.scalar.activation(
                            out=h1t[:, m * NT:(m + 1) * NT],
                            in_=pl[:],
                            func=mybir.ActivationFunctionType.Relu,
                        )

                # cur: lin-row transposes (all gathers are done by now)
                if cur is not None:
                    pltr = ptrp.tile([128, NT], dt.float32, tag="ptr")
                    for s in range(NSUB):
                        nc.tensor.matmul(
                            out=pltr[D:D + F, s * SUB:(s + 1) * SUB],
                            lhsT=C["gl"][:, s * F:(s + 1) * F],
                            rhs=identsb[:],
                            start=True,
                            stop=True,
                        )
                    nc.vector.tensor_copy(
                        out=C["cext"][D:D + F, :], in_=pltr[D:D + F, :]
                    )

                if P is None:
                    continue

                # ---- prev: layers 2/3 ----
                h2t = actp.tile([128, (H2 // 128) * NT], dt.bfloat16, tag="h2t")
                for m in range(H2 // 128):
                    pl = plp.tile([128, NT], dt.float32, tag="pl")
                    for c in range(H1 // 128):
                        nc.tensor.matmul(
                            out=pl[:],
                            lhsT=w2sb[:, c * H2 + m * 128:c * H2 + (m + 1) * 128],
                            rhs=h1t[:, c * NT:(c + 1) * NT],
                            start=(c == 0),
                            stop=(c == H1 // 128 - 1),
                        )
                    nc.scalar.activation(
                        out=h2t[:, m * NT:(m + 1) * NT],
                        in_=pl[:],
                        func=mybir.ActivationFunctionType.Relu,
                    )
                # FM second-order combine, overlaps L3 on PE
                asq = miscp.tile([D, NT], dt.float32, tag="asq")
                nc.scalar.activation(
                    out=asq[:], in_=pa[:], func=mybir.ActivationFunctionType.Square
                )
                nc.vector.tensor_tensor(
                    out=P["cext"][0:D, :], in0=asq[:], in1=pb[:],
                    op=mybir.AluOpType.subtract,
                )
                h3t = actp.tile([128, (H3 // 128) * NT], dt.bfloat16, tag="h3t")
                for m in range(H3 // 128):
                    pl = plp.tile([128, NT], dt.float32, tag="pl")
                    for c in range(H2 // 128):
                        nc.tensor.matmul(
                            out=pl[:],
                            lhsT=w3sb[:, c * H3 + m * 128:c * H3 + (m + 1) * 128],
                            rhs=h2t[:, c * NT:(c + 1) * NT],
                            start=(c == 0),
                            stop=(c == H2 // 128 - 1),
                        )
                    nc.scalar.activation(
                        out=h3t[:, m * NT:(m + 1) * NT],
                        in_=pl[:],
                        func=mybir.ActivationFunctionType.Relu,
                    )

                # ---- prev: final row ----
                pfin = pfinp.tile([1, NT], dt.float32)
                nc.tensor.matmul(
                    out=pfin[:],
                    lhsT=wldsb[:],
                    rhs=densesb[:, prev * NT:(prev + 1) * NT],
                    start=True,
                    stop=False,
                )
                for m in range(H3 // 128):
                    nc.tensor.matmul(
                        out=pfin[:],
                        lhsT=woutsb[:, m:m + 1],
                        rhs=h3t[:, m * NT:(m + 1) * NT],
                        start=False,
                        stop=False,
                    )
                nc.tensor.matmul(
                    out=pfin[:],
                    lhsT=coeffsb[0:KFIN, :],
                    rhs=P["cext"][0:KFIN, :],
                    start=False,
                    stop=True,
                )
                row = miscp.tile([1, NT], dt.float32, tag="row")
                nc.vector.tensor_copy(out=row[:], in_=pfin[:])
                nc.sync.dma_start(out=out[prev:prev + 1, :], in_=row[:])
                del H[prev]

    nc.finalize()
    return nc


def _prepare(dense, sparse_idx, bias, emb_tables, lin_tables, Wd, Wld, bld, W1, W2, W3, Wout):
    bf16 = ml_dtypes.bfloat16
    dense = np.asarray(dense, np.float32)
    sparse_idx = np.asarray(sparse_idx)
    table = np.zeros([FV, RSTRIDE], dtype=bf16)
    table[:, 0:D] = np.asarray(emb_tables, np.float32).reshape(FV, D).astype(bf16)
    table[:, D] = np.asarray(lin_tables, np.float32).reshape(FV).astype(bf16)
    wdcat = np.asarray(Wd, np.float32)
    wldv = np.asarray(Wld, np.float32).reshape(ND, 1)
    coeff = np.zeros([128, 1], np.float32)
    coeff[0:D, 0] = 0.5
    coeff[D:D + F, 0] = 1.0
    coeff[96, 0] = float(np.asarray(bias, np.float32).reshape(-1)[0]) + float(
        np.asarray(bld, np.float32).reshape(-1)[0]
    )
    stacki = np.tile(np.eye(D, dtype=bf16), (2, 1))
    ident = np.eye(128, dtype=bf16)
    off = (sparse_idx.astype(np.int64) + (np.arange(F, dtype=np.int64) * V)[None, :]).astype(np.int32)

    shared = {
        "table": table,
        "wdcat": wdcat.astype(np.float32),
        "wld": wldv,
        "w1": np.asarray(W1, np.float32).astype(bf16),
        "w2": np.asarray(W2, np.float32).astype(bf16),
        "w3": np.asarray(W3, np.float32).astype(bf16),
        "wout": np.asarray(Wout, np.float32).astype(bf16),
        "coeff": coeff,
        "stacki": stacki,
        "ident": ident,
    }
    in_maps = []
    for i in range(NCORES):
        sl = slice(i * BS, (i + 1) * BS)
        m = dict(shared)
        m["denseT"] = np.ascontiguousarray(dense[sl].T)
        m["idx"] = np.ascontiguousarray(off[sl])
        in_maps.append(m)
    return in_maps


def kernel(**inputs):
    from concourse.bass_utils import run_bass_kernel_spmd

    in_maps = _prepare(**inputs)
    if "nc" not in _cache:
        _cache["nc"] = _build_nc()
    res = run_bass_kernel_spmd(_cache["nc"], in_maps, list(range(NCORES)))
    outs = [r["out"].reshape(BS, 1).astype(np.float32) for r in res.results]
    return np.concatenate(outs, axis=0)

